# revision 23
# baseline (speedup 1.0000x reference)
"""ResNet BasicBlock forward on 8 Trainium2 NeuronCores.

Computes relu(bn2(conv2(relu(bn1(conv1(x))))) + x) for x[64,128,56,56],
two 3x3 stride-1 pad-1 convs with 128->128 channels, eval-mode BN.

Strategy:
  - Data parallel over batch: 8 images per core, no collectives.
  - Each 3x3 conv = shifted matmuls accumulated in PSUM. Input channels
    (128) sit on the SBUF partition dim (= matmul contraction dim); output
    channels land on PSUM partitions. Spatial output is tiled into 7 PSUM
    banks of 8 rows x 56 cols (448 fp32 = one 2KB bank). The all-zero pad
    row is trimmed off the edge banks' ky=0/ky=2 taps (ky=1 taps run
    first so the start=True matmul covers every psum row).
  - bf16 matmul inputs (1 cycle/row on the PE vs 4 for fp32), fp32 PSUM
    accumulation. conv1 additionally computes taps (ky=0, kx=0/1) as one
    fp8e4m3 DoubleRow matmul (2 MACs/cell/cycle; ~327ns vs 2x190ns) off a
    host-prepared fp8 copy of x with the two kx planes pre-shifted; conv1
    weights carry an exact-pow2 x32 scale (descaled by the epilogue's
    activation scale) to clear e4m3's subnormal range. Net rel err 0.0161
    (vs 0.0031 all-bf16), under the 2e-2 gate; verified bit-stable.
  - x is cast to bf16 AND zero-padded to 58x58 on the host, so every
    image input is one fully contiguous DMA straight into the padded SBUF
    buffer and every shifted tap is a plain strided access pattern. BN
    scale is folded into the conv weights; BN shift is a per-channel bias.
  - Epilogues: ScalarE does relu(psum/32 + b1) -> bf16 mid (padded);
    VectorE does (psum + b2) + residual then relu. The residual is read
    from the bf16 input buffer (adds <0.1% error).
  - Ramp: the first DMAs of a kernel take ~5.5us to complete regardless
    of size, so image 0's inputs are split across both HWDGE rings (xbf
    row-chunks on sync; w1 + fp8 copy + small tensors on scalar) and
    warmup matmuls bridge the wait with continuous PE activity — any
    multi-us PE idle resets the HAM clock-gate window and the real
    matmuls would then run at 1.2 GHz for ~3.4us.
  - Tail: the last image stores per-bank with the final bank split into
    two half-bank PSUM groups whose residual is folded into PSUM by an
    identity matmul, so each final epilogue is ONE op (ScalarE
    relu(psum+b2) / VectorE fused add+max) instead of a serial stt+max
    chain; DMAs alternate between the scalar and sync rings. This
    minimizes last-matmul -> final-DMA-receipt (~3.4us), which gates the
    fixed ~8us end-of-kernel semaphore-zeroing + engine-halt ladder. The
    Tile framework inserts all semaphores; images are software-pipelined
    DEPTH deep.
"""

import functools
import os
import sys

import numpy as np

for _p in ("/opt/trn_rl_repo", "/root/.axon_site/_ro/trn_rl_repo"):
    if os.path.isdir(_p) and _p not in sys.path:
        sys.path.append(_p)

import ml_dtypes  # noqa: E402

import concourse.bass as bass  # noqa: E402,F401
import concourse.mybir as mybir  # noqa: E402
import concourse.tile as tile  # noqa: E402
from concourse import bacc, bass_utils  # noqa: E402

N_CORES = 8
IMGS = 8  # images per core
C = 128
H = W = 56
HP = WP = 58  # padded spatial
RPB = 8  # output rows per PSUM bank
BANKS = H // RPB  # 7
KK = 9  # 3x3 taps
EPS = 1e-5
DEPTH = 4  # image pipeline depth
# The warmup must bridge user-start to first-input-ready with CONTINUOUS
# PE activity: any multi-us idle gap resets the HAM activity window and
# the real matmuls then run at 1.2 GHz for ~3.4us. Sized so the last
# warmup matmul ends just as image 0's first chunk lands (~12.4us on a
# typical run; the first-DMA burst can slip to ~15us, and 17 warmups keep
# the residual idle gap under the 3.4us HAM MID window either way).
WARM_MMS = 17
# Process ky=1 taps first within each PSUM accumulation group, so the
# start=True matmul covers every psum row even for the edge banks whose
# ky=0 / ky=2 taps are trimmed (their extra input row is all-zero pad).
TAP_ORDER = (3, 4, 5, 0, 1, 2, 6, 7, 8)
# conv1 computes taps 0 and 1 (the ky=0, kx=0/1 pair) as ONE fp8 DoubleRow
# matmul: the PE holds 2 fp8 weights per cell (virtual 128x256 array) and
# does 2 multiplies/cycle, so the pair costs ~1.13 matmuls instead of 2.
# Quantizing 2 of 18 conv-taps to e4m3 raises rel err 0.0031 -> 0.0161
# (measured; deterministic inputs), under the 2e-2 gate. conv1 weights are
# scaled x32 (exact pow2) so the fp8 weights clear the subnormal range;
# the conv1 epilogue descales via the activation's scale.
CONV1_BF16_TAPS = (3, 4, 5, 2, 6, 7, 8)  # ky=1 first (trim-safe start)
W1_SCALE = 32.0

BF16 = mybir.dt.bfloat16
F32 = mybir.dt.float32
F8 = mybir.dt.float8e4


def _build_module():
    nc = bacc.Bacc(
        "TRN2",
        target_bir_lowering=False,
        debug=False,
        enable_asserts=False,
        num_devices=N_CORES,
        enable_partition_id=False,
    )
    xbf_d = nc.dram_tensor("xbf", [IMGS, C, HP, WP], BF16, kind="ExternalInput").ap()
    x8_d = nc.dram_tensor("x8d", [IMGS, C, 2, HP, W], F8, kind="ExternalInput").ap()
    w18_d = nc.dram_tensor("w18", [C, 2, C], F8, kind="ExternalInput").ap()
    ident_d = nc.dram_tensor("ident", [C, C], BF16, kind="ExternalInput").ap()
    w1_d = nc.dram_tensor("w1t", [C, KK, C], BF16, kind="ExternalInput").ap()
    w2_d = nc.dram_tensor("w2t", [C, KK, C], BF16, kind="ExternalInput").ap()
    b1_d = nc.dram_tensor("b1", [C, 1], F32, kind="ExternalInput").ap()
    b2_d = nc.dram_tensor("b2", [C, 1], F32, kind="ExternalInput").ap()
    out_d = nc.dram_tensor("out", [IMGS, C, H, W], F32, kind="ExternalOutput").ap()

    add = mybir.AluOpType.add
    relu = mybir.ActivationFunctionType.Relu

    with tile.TileContext(nc) as tc:
        with (
            tc.tile_pool(name="singles", bufs=1) as singles,
            tc.tile_pool(name="psum", bufs=8, space="PSUM") as psum_pool,
        ):
            w1_sb = singles.tile([C, KK, C], BF16, name="w1_sb")
            w2_sb = singles.tile([C, KK, C], BF16, name="w2_sb")
            b1_sb = singles.tile([C, 1], F32, name="b1_sb")
            b2_sb = singles.tile([C, 1], F32, name="b2_sb")
            dummy = singles.tile([C, 1], F32, name="dummy")
            warm = singles.tile([C, 448], BF16, name="warm")
            w18_sb = singles.tile([C, 2, C], F8, name="w18_sb")
            ident_sb = singles.tile([C, C], BF16, name="ident_sb")

            x_pad = [
                singles.tile([C, HP, WP], BF16, name=f"x_pad{d}") for d in range(DEPTH)
            ]
            x8_pad = [
                singles.tile([C, 2, HP, W], F8, name=f"x8_pad{d}")
                for d in range(DEPTH)
            ]
            mid_pad = [
                singles.tile([C, HP, WP], BF16, name=f"mid_pad{d}")
                for d in range(DEPTH)
            ]
            out_sb = [
                singles.tile([C, H, W], F32, name=f"out_sb{d}") for d in range(DEPTH)
            ]

            # The warm tile feeds the throwaway warmup matmuls; memset it
            # first on VectorE so the PE can start immediately after the
            # framework preamble.
            nc.vector.memset(warm, 0.0)
            nc.vector.memset(dummy, 0.0)

            # Image 0's input lands in 3 row chunks on the sync HWDGE
            # queue (host pre-pads, so the transfer is contiguous); w1
            # rides the scalar queue concurrently so neither serializes
            # behind the other. First conv matmul needs w1 + chunk 0
            # (x_pad rows 0..17 cover output bank pair (0,1)).
            # Chunking x8 by rows would cut across its outer plane dim and
            # produce a slow 56B-segment strided DMA, so image 0's x8 goes
            # as ONE contiguous transfer, first on the scalar ring (the DR
            # matmuls that need it sit at the end of each tap group). w1
            # rides the sync ring between the xbf chunks.
            nc.scalar.dma_start(out=x8_pad[0], in_=x8_d[0])
            nc.scalar.dma_start(out=b1_sb, in_=b1_d)
            nc.scalar.dma_start(out=w18_sb, in_=w18_d)
            nc.sync.dma_start(
                out=x_pad[0][:, 0:18, :], in_=xbf_d[0][:, 0:18, :]
            )
            nc.sync.dma_start(out=w1_sb, in_=w1_d)
            for r0, r1 in ((18, 38), (38, HP)):
                nc.sync.dma_start(
                    out=x_pad[0][:, r0:r1, :], in_=xbf_d[0][:, r0:r1, :]
                )

            # Warm up the PE's HAM clock gate while image 0's DMA is in
            # flight: throwaway matmuls keep the PE busy so the activity
            # window opens and the real matmuls run at (or near) 2.4 GHz.
            wps = psum_pool.tile([C, 448], F32, name="ps")
            for wi in range(WARM_MMS):
                nc.tensor.matmul(
                    wps,
                    lhsT=warm[:, 0:C],
                    rhs=warm[:, :],
                    start=(wi == 0),
                    stop=(wi == WARM_MMS - 1),
                )

            # Hoist the ACT table load off the critical path: the first
            # ACTIVATE in the Scalar stream triggers it. Runs while image
            # 0's chunks transfer.
            nc.scalar.activation(out=dummy, in_=dummy, func=relu)
            nc.scalar.dma_start(out=b2_sb, in_=b2_d)
            nc.scalar.dma_start(out=w2_sb, in_=w2_d)
            nc.scalar.dma_start(out=ident_sb, in_=ident_d)

            # mid borders are never written by the per-image epilogues;
            # zero them once on GpSimd (otherwise idle, off critical path).
            for buf in mid_pad:
                nc.gpsimd.memset(buf[:, 0, :], 0.0)
                nc.gpsimd.memset(buf[:, HP - 1, :], 0.0)
                nc.gpsimd.memset(buf[:, 1 : HP - 1, 0 : WP : WP - 1], 0.0)

            for i in range(IMGS):
                d = i % DEPTH
                xp, mp, ob = x_pad[d], mid_pad[d], out_sb[d]
                x8p = x8_pad[d]
                if i > 0:
                    nc.sync.dma_start(out=xp, in_=xbf_d[i])
                    nc.scalar.dma_start(out=x8p, in_=x8_d[i])

                # Banks are processed in pairs sharing each tap's weights:
                # consecutive matmuls with the same stationary operand let
                # the weight load be reused/overlapped.
                pairs = [
                    tuple(b for b in (p, p + 1) if b < BANKS)
                    for p in range(0, BANKS, 2)
                ]

                # conv1 + bn1 + relu -> mid (bf16, padded)
                for pair in pairs:
                    pts = [psum_pool.tile([C, RPB, W], F32, name="ps") for _ in pair]
                    for ki, kk in enumerate(CONV1_BF16_TAPS):
                        ky, kx = divmod(kk, 3)
                        for ps, b in zip(pts, pair):
                            # Trim the all-zero pad row off the edge banks'
                            # vertical-shift taps (it contributes nothing).
                            lo = 1 if RPB * b + ky == 0 else 0
                            hi = RPB - (1 if RPB * b + ky + RPB == HP else 0)
                            nc.tensor.matmul(
                                ps[:, lo:hi, :],
                                lhsT=w1_sb[:, kk, :],
                                rhs=xp[
                                    :,
                                    RPB * b + ky + lo : RPB * b + ky + hi,
                                    kx : kx + W,
                                ],
                                start=(ki == 0),
                                stop=False,
                            )
                    for ps, b in zip(pts, pair):
                        # Taps 0,1 (ky=0) in one fp8 DoubleRow matmul; the
                        # host pre-shifted the two kx planes into x8_pad's
                        # dim-1 so rhs is [K, 2, rows, W].
                        lo = 1 if b == 0 else 0
                        nc.tensor.matmul(
                            ps[:, lo:RPB, :],
                            lhsT=w18_sb,
                            rhs=x8p[:, :, RPB * b + lo : RPB * b + RPB, :],
                            start=False,
                            stop=True,
                            perf_mode=mybir.MatmulPerfMode.DoubleRow,
                        )
                    for ps, b in zip(pts, pair):
                        nc.scalar.activation(
                            out=mp[:, 1 + RPB * b : 1 + RPB * (b + 1), 1 : W + 1],
                            in_=ps,
                            func=relu,
                            bias=b1_sb[:, 0:1],
                            scale=1.0 / W1_SCALE,
                        )

                # conv2 + bn2 + residual + relu -> out. Each group's
                # segments share tap weights. For the last image, the final
                # bank is split into two half-bank PSUM groups so the last
                # epilogue piece after the final matmul is small: its
                # stt+relu+DMA chain (plus the ~1.5us HBM write receipt the
                # end-of-kernel barrier waits on) is the serial tail.
                if i == IMGS - 1:
                    groups = [
                        ((0, 0, RPB), (1, 0, RPB)),
                        ((2, 0, RPB), (3, 0, RPB)),
                        ((4, 0, RPB), (5, 0, RPB)),
                        ((6, 0, RPB // 2), (6, RPB // 2, RPB)),
                    ]
                else:
                    groups = [((b, 0, RPB), (b + 1, 0, RPB)) for b in (0, 2, 4)]
                    groups.append(((6, 0, RPB),))
                seg_idx = 0
                for gi, group in enumerate(groups):
                    # For the very last group, fold the residual into PSUM
                    # with an identity matmul per segment, so the epilogue
                    # after the final matmul is a single op + DMA instead
                    # of a serial stt+max chain on the DVE.
                    id_res = i == IMGS - 1 and gi == len(groups) - 1
                    pts2 = [
                        psum_pool.tile([C, r1 - r0, W], F32, name="ps")
                        for (b, r0, r1) in group
                    ]
                    for ki, kk in enumerate(TAP_ORDER):
                        ky, kx = divmod(kk, 3)
                        for ps2, (b, r0, r1) in zip(pts2, group):
                            lo = r0 + (1 if RPB * b + ky + r0 == 0 else 0)
                            hi = r1 - (1 if RPB * b + ky + r1 == HP else 0)
                            nc.tensor.matmul(
                                ps2[:, lo - r0 : hi - r0, :],
                                lhsT=w2_sb[:, kk, :],
                                rhs=mp[
                                    :,
                                    RPB * b + ky + lo : RPB * b + ky + hi,
                                    kx : kx + W,
                                ],
                                start=(ki == 0),
                                stop=(ki == KK - 1) and not id_res,
                            )
                    if id_res:
                        for ps2, (b, r0, r1) in zip(pts2, group):
                            nc.tensor.matmul(
                                ps2,
                                lhsT=ident_sb,
                                rhs=xp[
                                    :, 1 + RPB * b + r0 : 1 + RPB * b + r1, 1 : W + 1
                                ],
                                start=False,
                                stop=True,
                            )
                    for si, (ps2, (b, r0, r1)) in enumerate(zip(pts2, group)):
                        rows = ob[:, RPB * b + r0 : RPB * b + r1, :]
                        if id_res:
                            # PSUM already holds conv + residual.
                            if si == 0:
                                nc.scalar.activation(
                                    out=rows, in_=ps2, func=relu,
                                    bias=b2_sb[:, 0:1],
                                )
                            else:
                                nc.vector.tensor_scalar(
                                    out=rows, in0=ps2,
                                    scalar1=b2_sb[:, 0:1], scalar2=0.0,
                                    op0=add, op1=mybir.AluOpType.max,
                                )
                        else:
                            nc.vector.scalar_tensor_tensor(
                                out=rows,
                                in0=ps2,
                                scalar=b2_sb[:, 0:1],
                                in1=xp[
                                    :, 1 + RPB * b + r0 : 1 + RPB * b + r1, 1 : W + 1
                                ],
                                op0=add,
                                op1=add,
                            )
                            nc.vector.tensor_scalar_max(rows, rows, 0.0)
                        if i == IMGS - 1:
                            dst = out_d[i][:, RPB * b + r0 : RPB * b + r1, :]
                            if seg_idx % 2 == 0:
                                nc.scalar.dma_start(out=dst, in_=rows)
                            else:
                                nc.sync.dma_start(out=dst, in_=rows)
                        seg_idx += 1

                if i < IMGS - 1:
                    nc.scalar.dma_start(out=out_d[i], in_=ob)

    nc.compile()
    return nc


def _install_neff_cache():
    """Content-addressed on-disk cache for walrus NEFF compiles.

    The BIR JSON for this module is byte-identical across processes, so a
    fresh process can reuse the NEFF compiled by an earlier one instead of
    paying the multi-minute walrus compile again.
    """
    import hashlib
    import shutil

    from concourse import bass2jax, bass_utils as bu

    if getattr(bu, "_neff_cache_installed", False):
        return
    orig = bu.compile_bir_kernel
    cache_dir = "/var/tmp/bass_neff_cache"

    def cached(bir_json, tmpdir, neff_name="file.neff"):
        data = bir_json if isinstance(bir_json, bytes) else bir_json.encode()
        key = hashlib.sha256(data).hexdigest()
        cpath = os.path.join(cache_dir, key + ".neff")
        try:
            if os.path.exists(cpath):
                dst = os.path.join(tmpdir, neff_name)
                shutil.copy(cpath, dst)
                return dst
        except OSError:
            pass
        neff_path = orig(bir_json, tmpdir, neff_name)
        try:
            os.makedirs(cache_dir, exist_ok=True)
            tmp = cpath + f".tmp{os.getpid()}"
            shutil.copy(neff_path, tmp)
            os.replace(tmp, cpath)
        except OSError:
            pass
        return neff_path

    bu.compile_bir_kernel = cached
    bass2jax.compile_bir_kernel = cached
    bu._neff_cache_installed = True


@functools.lru_cache(maxsize=1)
def _get_module():
    _install_neff_cache()
    return _build_module()


def _prep_in_maps(inputs):
    f32 = np.float32
    x = np.asarray(inputs["x"], f32)
    w1 = np.asarray(inputs["w1"], f32)
    w2 = np.asarray(inputs["w2"], f32)
    gamma1 = np.asarray(inputs["gamma1"], f32)
    beta1 = np.asarray(inputs["beta1"], f32)
    mean1 = np.asarray(inputs["mean1"], f32)
    var1 = np.asarray(inputs["var1"], f32)
    gamma2 = np.asarray(inputs["gamma2"], f32)
    beta2 = np.asarray(inputs["beta2"], f32)
    mean2 = np.asarray(inputs["mean2"], f32)
    var2 = np.asarray(inputs["var2"], f32)

    a1 = gamma1 / np.sqrt(var1 + EPS)
    a2 = gamma2 / np.sqrt(var2 + EPS)
    # Fold BN scale into weights; transpose to [c_in, tap, c_out] for lhsT.
    # conv1's weights carry an extra x32 (exact pow2; descaled by the conv1
    # epilogue) so its fp8 tap-pair clears e4m3's subnormal range.
    w1s = w1 * a1[:, None, None, None] * W1_SCALE
    w1t = np.ascontiguousarray(
        np.transpose(w1s, (1, 2, 3, 0)).reshape(C, KK, C)
    ).astype(ml_dtypes.bfloat16)
    # Taps (ky=0, kx=0) and (ky=0, kx=1) as the DoubleRow fp8 pair.
    w18 = np.ascontiguousarray(np.transpose(w1s[:, :, 0, 0:2], (1, 2, 0))).astype(
        ml_dtypes.float8_e4m3
    )
    w2t = np.ascontiguousarray(
        np.transpose(w2 * a2[:, None, None, None], (1, 2, 3, 0)).reshape(C, KK, C)
    ).astype(ml_dtypes.bfloat16)
    b1 = np.ascontiguousarray((beta1 - mean1 * a1).reshape(C, 1).astype(f32))
    b2 = np.ascontiguousarray((beta2 - mean2 * a2).reshape(C, 1).astype(f32))

    # Cast to bf16 and zero-pad to 58x58 on the host so the device DMA is
    # one contiguous transfer per image.
    xbf = np.zeros((N_CORES * IMGS, C, HP, WP), ml_dtypes.bfloat16)
    xbf[:, :, 1 : H + 1, 1 : W + 1] = x.astype(ml_dtypes.bfloat16)
    # fp8 copy for the DoubleRow pair, with the two kx shifts pre-sliced
    # into dim 2 so the matmul rhs is a plain [K, 2, rows, W] slice.
    x8full = xbf.astype(ml_dtypes.float8_e4m3)
    x8d = np.ascontiguousarray(
        np.stack([x8full[:, :, :, j : j + W] for j in (0, 1)], axis=2)
    )
    ident = np.eye(C, dtype=ml_dtypes.bfloat16)
    return [
        {
            "xbf": xbf[IMGS * i : IMGS * (i + 1)],
            "x8d": x8d[IMGS * i : IMGS * (i + 1)],
            "w1t": w1t,
            "w18": w18,
            "w2t": w2t,
            "b1": b1,
            "b2": b2,
            "ident": ident,
        }
        for i in range(N_CORES)
    ]


def _run(inputs, trace=False):
    nc = _get_module()
    in_maps = _prep_in_maps(inputs)
    res = bass_utils.run_bass_kernel_spmd(
        nc, in_maps, core_ids=list(range(N_CORES)), trace=trace
    )
    out = np.concatenate([r["out"] for r in res.results], axis=0)
    return out.astype(np.float32), res


def kernel(**inputs):
    out, _ = _run(inputs, trace=False)
    return out


# revision 24
# speedup vs baseline: 1.0042x; 1.0042x over previous
"""ResNet BasicBlock forward on 8 Trainium2 NeuronCores.

Computes relu(bn2(conv2(relu(bn1(conv1(x))))) + x) for x[64,128,56,56],
two 3x3 stride-1 pad-1 convs with 128->128 channels, eval-mode BN.

Strategy:
  - Data parallel over batch: 8 images per core, no collectives.
  - Each 3x3 conv = shifted matmuls accumulated in PSUM. Input channels
    (128) sit on the SBUF partition dim (= matmul contraction dim); output
    channels land on PSUM partitions. Spatial output is tiled into 7 PSUM
    banks of 8 rows x 56 cols (448 fp32 = one 2KB bank). The all-zero pad
    row is trimmed off the edge banks' ky=0/ky=2 taps (ky=1 taps run
    first so the start=True matmul covers every psum row).
  - bf16 matmul inputs (1 cycle/row on the PE vs 4 for fp32), fp32 PSUM
    accumulation. conv1 additionally computes taps (ky=0, kx=0/1) as one
    fp8e4m3 DoubleRow matmul (2 MACs/cell/cycle; ~327ns vs 2x190ns) off a
    host-prepared fp8 copy of x with the two kx planes pre-shifted; conv1
    weights carry an exact-pow2 x32 scale (descaled by the epilogue's
    activation scale) to clear e4m3's subnormal range. Net rel err 0.0161
    (vs 0.0031 all-bf16), under the 2e-2 gate; verified bit-stable.
  - x is cast to bf16 AND zero-padded to 58x58 on the host, so every
    image input is one fully contiguous DMA straight into the padded SBUF
    buffer and every shifted tap is a plain strided access pattern. BN
    scale is folded into the conv weights; BN shift is a per-channel bias.
  - Epilogues: ScalarE does relu(psum/32 + b1) -> bf16 mid (padded);
    VectorE does (psum + b2) + residual then relu. The residual is read
    from the bf16 input buffer (adds <0.1% error).
  - Ramp: the first DMAs of a kernel take ~5.5us to complete regardless
    of size, so image 0's inputs are split across both HWDGE rings (xbf
    row-chunks on sync; w1 + fp8 copy + small tensors on scalar) and
    warmup matmuls bridge the wait with continuous PE activity — any
    multi-us PE idle resets the HAM clock-gate window and the real
    matmuls would then run at 1.2 GHz for ~3.4us.
  - Tail: the last image stores per-bank with the final bank split into
    two half-bank PSUM groups whose residual is folded into PSUM by an
    identity matmul, so each final epilogue is ONE op (ScalarE
    relu(psum+b2) / VectorE fused add+max) instead of a serial stt+max
    chain; DMAs alternate between the scalar and sync rings. This
    minimizes last-matmul -> final-DMA-receipt (~3.4us), which gates the
    fixed ~8us end-of-kernel semaphore-zeroing + engine-halt ladder. The
    Tile framework inserts all semaphores; images are software-pipelined
    DEPTH deep.
"""

import functools
import os
import sys

import numpy as np

for _p in ("/opt/trn_rl_repo", "/root/.axon_site/_ro/trn_rl_repo"):
    if os.path.isdir(_p) and _p not in sys.path:
        sys.path.append(_p)

import ml_dtypes  # noqa: E402

import concourse.bass as bass  # noqa: E402,F401
import concourse.mybir as mybir  # noqa: E402
import concourse.tile as tile  # noqa: E402
from concourse import bacc, bass_utils  # noqa: E402

N_CORES = 8
IMGS = 8  # images per core
C = 128
H = W = 56
HP = WP = 58  # padded spatial
RPB = 8  # output rows per PSUM bank
BANKS = H // RPB  # 7
KK = 9  # 3x3 taps
EPS = 1e-5
DEPTH = 4  # image pipeline depth
# The warmup must bridge user-start to first-input-ready with CONTINUOUS
# PE activity: any multi-us idle gap resets the HAM activity window and
# the real matmuls then run at 1.2 GHz for ~3.4us. Sized so the last
# warmup matmul ends just as image 0's first chunk lands (~12.4us on a
# typical run; the first-DMA burst can slip to ~15us, and 17 warmups keep
# the residual idle gap under the 3.4us HAM MID window either way).
WARM_MMS = 17
# Process ky=1 taps first within each PSUM accumulation group, so the
# start=True matmul covers every psum row even for the edge banks whose
# ky=0 / ky=2 taps are trimmed (their extra input row is all-zero pad).
TAP_ORDER = (3, 4, 5, 0, 1, 2, 6, 7, 8)
# conv1 computes taps 0 and 1 (the ky=0, kx=0/1 pair) as ONE fp8 DoubleRow
# matmul: the PE holds 2 fp8 weights per cell (virtual 128x256 array) and
# does 2 multiplies/cycle, so the pair costs ~1.13 matmuls instead of 2.
# Quantizing 2 of 18 conv-taps to e4m3 raises rel err 0.0031 -> 0.0161
# (measured; deterministic inputs), under the 2e-2 gate. conv1 weights are
# scaled x32 (exact pow2) so the fp8 weights clear the subnormal range;
# the conv1 epilogue descales via the activation's scale.
CONV1_BF16_TAPS = (3, 4, 5, 2, 6, 7, 8)  # ky=1 first (trim-safe start)
W1_SCALE = 32.0

BF16 = mybir.dt.bfloat16
F32 = mybir.dt.float32
F8 = mybir.dt.float8e4


def _build_module():
    nc = bacc.Bacc(
        "TRN2",
        target_bir_lowering=False,
        debug=False,
        enable_asserts=False,
        num_devices=N_CORES,
        enable_partition_id=False,
    )
    xbf_d = nc.dram_tensor("xbf", [IMGS, C, HP, WP], BF16, kind="ExternalInput").ap()
    x8_d = nc.dram_tensor("x8d", [IMGS, C, 2, HP, W], F8, kind="ExternalInput").ap()
    w18_d = nc.dram_tensor("w18", [C, 2, C], F8, kind="ExternalInput").ap()
    ident_d = nc.dram_tensor("ident", [C, C], BF16, kind="ExternalInput").ap()
    w1_d = nc.dram_tensor("w1t", [C, KK, C], BF16, kind="ExternalInput").ap()
    w2_d = nc.dram_tensor("w2t", [C, KK, C], BF16, kind="ExternalInput").ap()
    b1_d = nc.dram_tensor("b1", [C, 1], F32, kind="ExternalInput").ap()
    b2_d = nc.dram_tensor("b2", [C, 1], F32, kind="ExternalInput").ap()
    out_d = nc.dram_tensor("out", [IMGS, C, H, W], F32, kind="ExternalOutput").ap()

    add = mybir.AluOpType.add
    relu = mybir.ActivationFunctionType.Relu

    with tile.TileContext(nc) as tc:
        with (
            tc.tile_pool(name="singles", bufs=1) as singles,
            tc.tile_pool(name="psum", bufs=8, space="PSUM") as psum_pool,
        ):
            w1_sb = singles.tile([C, KK, C], BF16, name="w1_sb")
            w2_sb = singles.tile([C, KK, C], BF16, name="w2_sb")
            b1_sb = singles.tile([C, 1], F32, name="b1_sb")
            b2_sb = singles.tile([C, 1], F32, name="b2_sb")
            dummy = singles.tile([C, 1], F32, name="dummy")
            warm = singles.tile([C, 448], BF16, name="warm")
            w18_sb = singles.tile([C, 2, C], F8, name="w18_sb")
            ident_sb = singles.tile([C, C], BF16, name="ident_sb")

            x_pad = [
                singles.tile([C, HP, WP], BF16, name=f"x_pad{d}") for d in range(DEPTH)
            ]
            x8_pad = [
                singles.tile([C, 2, HP, W], F8, name=f"x8_pad{d}")
                for d in range(DEPTH)
            ]
            mid_pad = [
                singles.tile([C, HP, WP], BF16, name=f"mid_pad{d}")
                for d in range(DEPTH)
            ]
            out_sb = [
                singles.tile([C, H, W], F32, name=f"out_sb{d}") for d in range(DEPTH)
            ]

            # The warm tile feeds the throwaway warmup matmuls; memset it
            # first on VectorE so the PE can start immediately after the
            # framework preamble.
            nc.vector.memset(warm, 0.0)
            nc.vector.memset(dummy, 0.0)

            # Image 0's input lands in 3 row chunks on the sync HWDGE
            # queue (host pre-pads, so the transfer is contiguous); w1
            # rides the scalar queue concurrently so neither serializes
            # behind the other. First conv matmul needs w1 + chunk 0
            # (x_pad rows 0..17 cover output bank pair (0,1)).
            # Chunking x8 by rows would cut across its outer plane dim and
            # produce a slow 56B-segment strided DMA, so image 0's x8 goes
            # as ONE contiguous transfer, first on the scalar ring (the DR
            # matmuls that need it sit at the end of each tap group). w1
            # rides the sync ring between the xbf chunks.
            nc.scalar.dma_start(out=x8_pad[0], in_=x8_d[0])
            nc.scalar.dma_start(out=b1_sb, in_=b1_d)
            nc.scalar.dma_start(out=w18_sb, in_=w18_d)
            nc.sync.dma_start(
                out=x_pad[0][:, 0:18, :], in_=xbf_d[0][:, 0:18, :]
            )
            nc.sync.dma_start(out=w1_sb, in_=w1_d)
            for r0, r1 in ((18, 38), (38, HP)):
                nc.sync.dma_start(
                    out=x_pad[0][:, r0:r1, :], in_=xbf_d[0][:, r0:r1, :]
                )

            # Warm up the PE's HAM clock gate while image 0's DMA is in
            # flight: throwaway matmuls keep the PE busy so the activity
            # window opens and the real matmuls run at (or near) 2.4 GHz.
            wps = psum_pool.tile([C, 448], F32, name="ps")
            for wi in range(WARM_MMS):
                nc.tensor.matmul(
                    wps,
                    lhsT=warm[:, 0:C],
                    rhs=warm[:, :],
                    start=(wi == 0),
                    stop=(wi == WARM_MMS - 1),
                )

            # Hoist the ACT table load off the critical path: the first
            # ACTIVATE in the Scalar stream triggers it. Runs while image
            # 0's chunks transfer.
            nc.scalar.activation(out=dummy, in_=dummy, func=relu)
            nc.scalar.dma_start(out=b2_sb, in_=b2_d)
            nc.scalar.dma_start(out=w2_sb, in_=w2_d)
            nc.scalar.dma_start(out=ident_sb, in_=ident_d)

            # mid borders are never written by the per-image epilogues;
            # zero them once on GpSimd (otherwise idle, off critical path).
            for buf in mid_pad:
                nc.gpsimd.memset(buf[:, 0, :], 0.0)
                nc.gpsimd.memset(buf[:, HP - 1, :], 0.0)
                nc.gpsimd.memset(buf[:, 1 : HP - 1, 0 : WP : WP - 1], 0.0)

            for i in range(IMGS):
                d = i % DEPTH
                xp, mp, ob = x_pad[d], mid_pad[d], out_sb[d]
                x8p = x8_pad[d]
                if i > 0:
                    nc.sync.dma_start(out=xp, in_=xbf_d[i])
                    nc.scalar.dma_start(out=x8p, in_=x8_d[i])

                # Banks are processed in pairs sharing each tap's weights:
                # consecutive matmuls with the same stationary operand let
                # the weight load be reused/overlapped.
                pairs = [
                    tuple(b for b in (p, p + 1) if b < BANKS)
                    for p in range(0, BANKS, 2)
                ]

                # conv1 + bn1 + relu -> mid (bf16, padded)
                for pair in pairs:
                    pts = [psum_pool.tile([C, RPB, W], F32, name="ps") for _ in pair]
                    for ki, kk in enumerate(CONV1_BF16_TAPS):
                        ky, kx = divmod(kk, 3)
                        for ps, b in zip(pts, pair):
                            # Trim the all-zero pad row off the edge banks'
                            # vertical-shift taps (it contributes nothing).
                            lo = 1 if RPB * b + ky == 0 else 0
                            hi = RPB - (1 if RPB * b + ky + RPB == HP else 0)
                            nc.tensor.matmul(
                                ps[:, lo:hi, :],
                                lhsT=w1_sb[:, kk, :],
                                rhs=xp[
                                    :,
                                    RPB * b + ky + lo : RPB * b + ky + hi,
                                    kx : kx + W,
                                ],
                                start=(ki == 0),
                                stop=False,
                            )
                    for ps, b in zip(pts, pair):
                        # Taps 0,1 (ky=0) in one fp8 DoubleRow matmul; the
                        # host pre-shifted the two kx planes into x8_pad's
                        # dim-1 so rhs is [K, 2, rows, W].
                        lo = 1 if b == 0 else 0
                        nc.tensor.matmul(
                            ps[:, lo:RPB, :],
                            lhsT=w18_sb,
                            rhs=x8p[:, :, RPB * b + lo : RPB * b + RPB, :],
                            start=False,
                            stop=True,
                            perf_mode=mybir.MatmulPerfMode.DoubleRow,
                        )
                    for ps, b in zip(pts, pair):
                        nc.scalar.activation(
                            out=mp[:, 1 + RPB * b : 1 + RPB * (b + 1), 1 : W + 1],
                            in_=ps,
                            func=relu,
                            bias=b1_sb[:, 0:1],
                            scale=1.0 / W1_SCALE,
                        )

                # conv2 + bn2 + residual + relu -> out. Each group's
                # segments share tap weights. For the last image, the final
                # bank is split into two half-bank PSUM groups so the last
                # epilogue piece after the final matmul is small: its
                # stt+relu+DMA chain (plus the ~1.5us HBM write receipt the
                # end-of-kernel barrier waits on) is the serial tail.
                if i == IMGS - 1:
                    groups = [
                        ((0, 0, RPB), (1, 0, RPB)),
                        ((2, 0, RPB), (3, 0, RPB)),
                        ((4, 0, RPB), (5, 0, RPB)),
                        ((6, 0, 6), (6, 6, RPB)),
                    ]
                else:
                    groups = [((b, 0, RPB), (b + 1, 0, RPB)) for b in (0, 2, 4)]
                    groups.append(((6, 0, RPB),))
                seg_idx = 0
                for gi, group in enumerate(groups):
                    # For the very last group, fold the residual into PSUM
                    # with an identity matmul per segment, so the epilogue
                    # after the final matmul is a single op + DMA instead
                    # of a serial stt+max chain on the DVE.
                    id_res = i == IMGS - 1 and gi == len(groups) - 1
                    pts2 = [
                        psum_pool.tile([C, r1 - r0, W], F32, name="ps")
                        for (b, r0, r1) in group
                    ]
                    for ki, kk in enumerate(TAP_ORDER):
                        ky, kx = divmod(kk, 3)
                        for ps2, (b, r0, r1) in zip(pts2, group):
                            lo = r0 + (1 if RPB * b + ky + r0 == 0 else 0)
                            hi = r1 - (1 if RPB * b + ky + r1 == HP else 0)
                            nc.tensor.matmul(
                                ps2[:, lo - r0 : hi - r0, :],
                                lhsT=w2_sb[:, kk, :],
                                rhs=mp[
                                    :,
                                    RPB * b + ky + lo : RPB * b + ky + hi,
                                    kx : kx + W,
                                ],
                                start=(ki == 0),
                                stop=(ki == KK - 1) and not id_res,
                            )
                    if id_res:
                        for ps2, (b, r0, r1) in zip(pts2, group):
                            nc.tensor.matmul(
                                ps2,
                                lhsT=ident_sb,
                                rhs=xp[
                                    :, 1 + RPB * b + r0 : 1 + RPB * b + r1, 1 : W + 1
                                ],
                                start=False,
                                stop=True,
                            )
                    for si, (ps2, (b, r0, r1)) in enumerate(zip(pts2, group)):
                        rows = ob[:, RPB * b + r0 : RPB * b + r1, :]
                        if id_res:
                            # PSUM already holds conv + residual.
                            if si == 0:
                                nc.scalar.activation(
                                    out=rows, in_=ps2, func=relu,
                                    bias=b2_sb[:, 0:1],
                                )
                            else:
                                nc.vector.tensor_scalar(
                                    out=rows, in0=ps2,
                                    scalar1=b2_sb[:, 0:1], scalar2=0.0,
                                    op0=add, op1=mybir.AluOpType.max,
                                )
                        else:
                            nc.vector.scalar_tensor_tensor(
                                out=rows,
                                in0=ps2,
                                scalar=b2_sb[:, 0:1],
                                in1=xp[
                                    :, 1 + RPB * b + r0 : 1 + RPB * b + r1, 1 : W + 1
                                ],
                                op0=add,
                                op1=add,
                            )
                            nc.vector.tensor_scalar_max(rows, rows, 0.0)
                        if i == IMGS - 1:
                            dst = out_d[i][:, RPB * b + r0 : RPB * b + r1, :]
                            if seg_idx % 2 == 0:
                                nc.scalar.dma_start(out=dst, in_=rows)
                            else:
                                nc.sync.dma_start(out=dst, in_=rows)
                        seg_idx += 1

                if i < IMGS - 1:
                    nc.scalar.dma_start(out=out_d[i], in_=ob)

    nc.compile()
    return nc


def _install_neff_cache():
    """Content-addressed on-disk cache for walrus NEFF compiles.

    The BIR JSON for this module is byte-identical across processes, so a
    fresh process can reuse the NEFF compiled by an earlier one instead of
    paying the multi-minute walrus compile again.
    """
    import hashlib
    import shutil

    from concourse import bass2jax, bass_utils as bu

    if getattr(bu, "_neff_cache_installed", False):
        return
    orig = bu.compile_bir_kernel
    cache_dir = "/var/tmp/bass_neff_cache"

    def cached(bir_json, tmpdir, neff_name="file.neff"):
        data = bir_json if isinstance(bir_json, bytes) else bir_json.encode()
        key = hashlib.sha256(data).hexdigest()
        cpath = os.path.join(cache_dir, key + ".neff")
        try:
            if os.path.exists(cpath):
                dst = os.path.join(tmpdir, neff_name)
                shutil.copy(cpath, dst)
                return dst
        except OSError:
            pass
        neff_path = orig(bir_json, tmpdir, neff_name)
        try:
            os.makedirs(cache_dir, exist_ok=True)
            tmp = cpath + f".tmp{os.getpid()}"
            shutil.copy(neff_path, tmp)
            os.replace(tmp, cpath)
        except OSError:
            pass
        return neff_path

    bu.compile_bir_kernel = cached
    bass2jax.compile_bir_kernel = cached
    bu._neff_cache_installed = True


@functools.lru_cache(maxsize=1)
def _get_module():
    _install_neff_cache()
    return _build_module()


def _prep_in_maps(inputs):
    f32 = np.float32
    x = np.asarray(inputs["x"], f32)
    w1 = np.asarray(inputs["w1"], f32)
    w2 = np.asarray(inputs["w2"], f32)
    gamma1 = np.asarray(inputs["gamma1"], f32)
    beta1 = np.asarray(inputs["beta1"], f32)
    mean1 = np.asarray(inputs["mean1"], f32)
    var1 = np.asarray(inputs["var1"], f32)
    gamma2 = np.asarray(inputs["gamma2"], f32)
    beta2 = np.asarray(inputs["beta2"], f32)
    mean2 = np.asarray(inputs["mean2"], f32)
    var2 = np.asarray(inputs["var2"], f32)

    a1 = gamma1 / np.sqrt(var1 + EPS)
    a2 = gamma2 / np.sqrt(var2 + EPS)
    # Fold BN scale into weights; transpose to [c_in, tap, c_out] for lhsT.
    # conv1's weights carry an extra x32 (exact pow2; descaled by the conv1
    # epilogue) so its fp8 tap-pair clears e4m3's subnormal range.
    w1s = w1 * a1[:, None, None, None] * W1_SCALE
    w1t = np.ascontiguousarray(
        np.transpose(w1s, (1, 2, 3, 0)).reshape(C, KK, C)
    ).astype(ml_dtypes.bfloat16)
    # Taps (ky=0, kx=0) and (ky=0, kx=1) as the DoubleRow fp8 pair.
    w18 = np.ascontiguousarray(np.transpose(w1s[:, :, 0, 0:2], (1, 2, 0))).astype(
        ml_dtypes.float8_e4m3
    )
    w2t = np.ascontiguousarray(
        np.transpose(w2 * a2[:, None, None, None], (1, 2, 3, 0)).reshape(C, KK, C)
    ).astype(ml_dtypes.bfloat16)
    b1 = np.ascontiguousarray((beta1 - mean1 * a1).reshape(C, 1).astype(f32))
    b2 = np.ascontiguousarray((beta2 - mean2 * a2).reshape(C, 1).astype(f32))

    # Cast to bf16 and zero-pad to 58x58 on the host so the device DMA is
    # one contiguous transfer per image.
    xbf = np.zeros((N_CORES * IMGS, C, HP, WP), ml_dtypes.bfloat16)
    xbf[:, :, 1 : H + 1, 1 : W + 1] = x.astype(ml_dtypes.bfloat16)
    # fp8 copy for the DoubleRow pair, with the two kx shifts pre-sliced
    # into dim 2 so the matmul rhs is a plain [K, 2, rows, W] slice.
    x8full = xbf.astype(ml_dtypes.float8_e4m3)
    x8d = np.ascontiguousarray(
        np.stack([x8full[:, :, :, j : j + W] for j in (0, 1)], axis=2)
    )
    ident = np.eye(C, dtype=ml_dtypes.bfloat16)
    return [
        {
            "xbf": xbf[IMGS * i : IMGS * (i + 1)],
            "x8d": x8d[IMGS * i : IMGS * (i + 1)],
            "w1t": w1t,
            "w18": w18,
            "w2t": w2t,
            "b1": b1,
            "b2": b2,
            "ident": ident,
        }
        for i in range(N_CORES)
    ]


def _run(inputs, trace=False):
    nc = _get_module()
    in_maps = _prep_in_maps(inputs)
    res = bass_utils.run_bass_kernel_spmd(
        nc, in_maps, core_ids=list(range(N_CORES)), trace=trace
    )
    out = np.concatenate([r["out"] for r in res.results], axis=0)
    return out.astype(np.float32), res


def kernel(**inputs):
    out, _ = _run(inputs, trace=False)
    return out
